# revision 17
# baseline (speedup 1.0000x reference)
"""Trainium2 Bass kernel for nn_MetricNet (512-step elementwise Euler recurrence).

Strategy: pure data parallel over the batch axis — each of the 8 NeuronCores
gets 16384 frequencies laid out as a [128 partitions x 128 free] f32 tile that
lives in SBUF for the whole 512-step recurrence.

Per-step schedule (vs the 4-DVE + 3-ACT baseline at ~1740 ns/step, this runs
~1310 ns/step):

  DVE : T1  = (Ys + cb)*U              [stt]          (cb = c1+beta)
        v2' = (T1 + 2*kt)*T1           [stt == (T1+kt)^2 - kt^2]
        gg  = (v2' - (S-kt^2))*W       [stt]
        Ys' = (a2 - cb^2/2) - gg       [stt]
  ACT : a2 = Square(r2*Ys + r2*cb)     [== (Ys+cb)^2/2, bias table]
        U' = Identity(T1 + ktd)        [bias table]

The critical dependency cycle (T1 -> v2' -> gg -> Ys') runs as four
back-to-back DVE instructions with no cross-engine hop — the DVE is 100%
busy and every other engine feeds it off-cycle. The square is an stt via
(T1+2kt)*T1 with the kt^2 correction folded into S; the quadratic Y-term
arrives via the ACT's a2 (computed from Ys at the top of the step, half a
cycle ahead of its use; its beta-corrections cancel exactly against the
-cb^2/2 immediate). The Pool/GpSimd engine is unused in the loop (its SBUF
traffic measurably slows concurrent DVE instructions).

The U-state update absorbs the inv1-shift schedule (ktd = kt + delta with
delta_last = -inv1_last so the final U IS Re_out); the sigma source term
rides a host-tracked beta offset on Ys (beta' = c1*beta + beta^2/2 + sigma)
folded into per-step immediates. All per-step scalars are host-precomputed
in float64; the only SBUF tables are the two ACT bias columns (interleaved,
split-DMA'd so the first steps' columns land early). U0/Y0/W are prepared
on the host so the loop starts straight off the DMA; the final
Im = (Ys_N + beta_N)/m rescale runs on the host during unsharding.
"""

import numpy as np

import concourse.bass as bass
import concourse.mybir as mybir
import bass_rust as _br
from concourse import tile
from concourse.bass_utils import run_bass_kernel_spmd

# walrus's codegen rejects instructions carrying more than ~2 sync-wait
# commands, but Tile's exit path hangs the full end-of-kernel wait set
# (one per engine/DMA lane used) on a single SP drain. Split those waits
# across dedicated one-wait NOPs ahead of a bare drain instead.
_orig_drain_and_barrier = tile.TileContext._drain_and_barrier


def _split_drain_and_barrier(self, tick_clock, wait_clock):
    nc = self.nc
    probe = nc.sync.nop()
    wait_clock.add_sem_waits(
        probe.ins, _br.ScopedClock({None: tick_clock.global_clock})
    )
    si = probe.ins.sync_info
    if si is not None and len(si.on_wait) > 1:
        waits = list(si.on_wait)
        probe.ins.sync_info = _br.SyncInfo(
            on_wait=waits[:1], on_update=list(si.on_update)
        )
        for w in waits[1:]:
            extra = nc.sync.nop()
            extra.ins.sync_info = _br.SyncInfo(on_wait=[w], on_update=[])
    nc.sync.drain()
    nc.all_engine_barrier()
    popped = nc._tile_sem_poison_stack.pop()
    assert popped is self._sem_poison
    nc.clear_and_free_semaphores(list(self.sems.allocated().values()))
    nc.all_engine_barrier()


tile.TileContext._drain_and_barrier = _split_drain_and_barrier


def _hoist_extra_waits(nc):
    """walrus's per-instruction sync-wait budget is 1 for compute/DMA
    instructions (2 for TPB_CTRL). Hoist surplus waits onto same-engine NOPs
    spliced immediately before the over-budget instruction — the engine
    executes in order, so waiting earlier is semantically identical."""
    for bb in nc.main_func.blocks:
        insts = bb.instructions
        out = []
        changed = False
        for ins in insts:
            si = ins.sync_info
            if si is not None and len(si.on_wait) > 1:
                waits = list(si.on_wait)
                for w in waits[:-1]:
                    nop = mybir.InstNoOp(
                        name=nc.get_next_instruction_name(),
                        engine=ins.engine,
                        sync_info=_br.SyncInfo(on_wait=[w], on_update=[]),
                    )
                    nc.register_instruction(nop)
                    out.append(nop)
                ins.sync_info = _br.SyncInfo(
                    on_wait=waits[-1:], on_update=list(si.on_update)
                )
                changed = True
            out.append(ins)
        if changed:
            bb.instructions = out


N_LAYERS = 512
Z_INI = 0.0
DEL_Z = 0.9 / 512.0
MU = 1.0
BATCH = 131072
N_CORES = 8
P = 128
F = BATCH // N_CORES // P  # 128

F32 = mybir.dt.float32
ALU = mybir.AluOpType
SQ = mybir.ActivationFunctionType.Square


def _host_scalars(B: np.ndarray, p: float):
    """Per-step scalar schedule, float64."""
    n = N_LAYERS
    zs = Z_INI + DEL_Z * np.arange(n, dtype=np.float64)
    b1 = B.astype(np.float64)[:n]
    b2 = B.astype(np.float64)[1 : n + 1]
    c1 = 2.0 - b2 / b1  # 1 + g
    inv1 = 1.0 / (p * (1.0 - zs))
    inv2 = inv1 / (1.0 - zs)
    kt = -DEL_Z * inv2
    delta = np.empty(n)
    delta[:-1] = inv1[1:] - inv1[:-1]
    delta[-1] = -inv1[-1]  # so the final U update yields Re_out exactly
    ktd = kt + delta
    S = -inv2 / p + inv1**2 + 1.0 / b1**2 - kt * kt
    sigma = -2.0 * DEL_Z * DEL_Z * zs * zs * (MU * MU) / b1
    beta = np.zeros(n + 1)
    for j in range(n):
        beta[j + 1] = c1[j] * beta[j] + 0.5 * beta[j] * beta[j] + sigma[j]
    return c1, kt, ktd, S, beta, inv1


def _build_bass(c1, kt, ktd, S, beta, inv1):
    n = N_LAYERS
    nc = bass.Bass()
    # packed input: [U0 | Y0 | W | a2-bias-table | ktd-table] (host-prepared)
    x_in = nc.dram_tensor("x_in", [P, 3 * F + 2 * n], F32, kind="ExternalInput")
    # packed output: [Re_out | Ys_final] (host rescales Ys -> Im)
    x_out = nc.dram_tensor("x_out", [P, 2 * F], F32, kind="ExternalOutput")

    f = float  # immediates
    with tile.TileContext(nc) as tc:
        with tc.tile_pool(name="pool", bufs=1) as pool:
            dummy = pool.tile([P, 1], F32)
            # trigger the ACT function-table load during the input DMA
            nc.scalar.activation(
                dummy[:], nc.const_aps.aps[(F32, 0.0)], SQ
            )
            xin = pool.tile([P, 3 * F + 2 * n], F32)
            tb = 3 * F
            head = min(128, 2 * n)
            nc.sync.dma_start(xin[:, 0 : 2 * F], x_in[:, 0 : 2 * F])
            nc.sync.dma_start(xin[:, 2 * F : tb], x_in[:, 2 * F : tb])
            # bias tables interleaved [a2_0, ktd_0, a2_1, ktd_1, ...]; split
            # so the first steps' columns land fast while the bulk streams in
            nc.scalar.dma_start(
                xin[:, tb : tb + head], x_in[:, tb : tb + head]
            )
            if 2 * n > head:
                nc.scalar.dma_start(
                    xin[:, tb + head : tb + 2 * n],
                    x_in[:, tb + head : tb + 2 * n],
                )
            U0 = xin[:, 0:F]
            Y0 = xin[:, F : 2 * F]
            W = xin[:, 2 * F : 3 * F]

            Ya = pool.tile([P, F], F32)
            Yb = pool.tile([P, F], F32)
            a2a = pool.tile([P, F], F32)
            a2b = pool.tile([P, F], F32)
            Ua = pool.tile([P, F], F32)
            Ub = pool.tile([P, F], F32)
            Ta = pool.tile([P, F], F32)
            Tb = pool.tile([P, F], F32)
            v2 = pool.tile([P, F], F32)
            gg = pool.tile([P, F], F32)
            xout = pool.tile([P, 2 * F], F32)
            reo = xout[:, 0:F]
            imo = xout[:, F : 2 * F]

            v = nc.vector
            stt = v.scalar_tensor_tensor
            Ys, Yn = Ya, Yb
            U, Un = Ua, Ub
            T1, T1n = Ta, Tb
            a2, a2n = a2a, a2b
            ID = mybir.ActivationFunctionType.Identity
            R2 = float(1.0 / np.sqrt(2.0))
            for j in range(n):
                cb = c1[j] + beta[j]
                ys_src = Y0 if j == 0 else Ys[:]
                u_src = U0 if j == 0 else U[:]
                yn_dst = imo if j == n - 1 else Yn[:]
                nc.scalar.activation(
                    a2[:], ys_src, SQ,
                    bias=xin[:, tb + 2 * j : tb + 2 * j + 1], scale=R2,
                )
                stt(T1[:], ys_src, f(cb), u_src, ALU.add, ALU.mult)
                stt(v2[:], T1[:], f(2.0 * kt[j]), T1[:], ALU.add, ALU.mult)
                un_dst = reo if j == n - 1 else Un[:]
                nc.scalar.activation(
                    un_dst, T1[:], ID,
                    bias=xin[:, tb + 2 * j + 1 : tb + 2 * j + 2],
                )
                stt(gg[:], v2[:], f(S[j]), W, ALU.subtract, ALU.mult)
                stt(yn_dst, a2[:], f(-0.5 * cb * cb), gg[:], ALU.add, ALU.subtract)
                Ys, Yn = Yn, Ys
                U, Un = Un, U
                T1, T1n = T1n, T1
                a2, a2n = a2n, a2

            nc.sync.dma_start(x_out[:], xout[:])
    _hoist_extra_waits(nc)
    return nc


def kernel(Re_s, Im_s, omega, PiT, B, _trace=False):
    Re_s = np.ascontiguousarray(Re_s, dtype=np.float32)
    Im_s = np.ascontiguousarray(Im_s, dtype=np.float32)
    omega = np.ascontiguousarray(omega, dtype=np.float32)
    p = float(np.asarray(PiT).reshape(-1)[0])
    c1, kt, ktd, S, beta, inv1 = _host_scalars(np.asarray(B), p)

    nc = _build_bass(c1, kt, ktd, S, beta, inv1)

    m64 = 2.0 * DEL_Z * omega.astype(np.float64)
    U0 = (Re_s.astype(np.float64) + inv1[0]).astype(np.float32)
    Y0 = (Im_s.astype(np.float64) * m64).astype(np.float32)
    Wf = (0.5 * m64 * m64).astype(np.float32)
    U08 = U0.reshape(N_CORES, P, F)
    Y08 = Y0.reshape(N_CORES, P, F)
    W8 = Wf.reshape(N_CORES, P, F)
    cb = c1 + beta[:N_LAYERS]
    tabs = np.empty(2 * N_LAYERS)
    tabs[0::2] = cb / np.sqrt(2.0)
    tabs[1::2] = ktd
    tabs = tabs.astype(np.float32)  # interleaved ACT bias tables
    tab8 = np.broadcast_to(tabs, (N_CORES, P, tabs.size))
    xin = np.concatenate([U08, Y08, W8, tab8], axis=2)  # [8, P, 3F+2n]
    in_maps = [{"x_in": np.ascontiguousarray(xin[i])} for i in range(N_CORES)]
    res = run_bass_kernel_spmd(nc, in_maps, list(range(N_CORES)), trace=_trace)
    re_full = np.concatenate(
        [res.results[i]["x_out"][:, 0:F].reshape(-1) for i in range(N_CORES)]
    )
    ys_full = np.concatenate(
        [res.results[i]["x_out"][:, F : 2 * F].reshape(-1) for i in range(N_CORES)]
    )
    im_full = (ys_full.astype(np.float64) + beta[N_LAYERS]) / m64
    if _trace:
        kernel.last_results = res
    return re_full.astype(np.float32), im_full.astype(np.float32)
